# revision 6
# baseline (speedup 1.0000x reference)
"""Multi-head attention (B=2, S=2048, D=1024, H=16) on 8 TRN2 NeuronCores.

Sharding: core c -> (batch b = c//4, head group g = c%4) — 4 heads/core
(tensor parallel on heads x data parallel on batch). Weight slices are
pre-transposed on the host so every device DMA is natural layout; the
w_o partial-sum reduction across each batch's 4 cores happens at gather.
"""

import numpy as np

import concourse.bass as bass
import concourse.bacc as bacc
import concourse.mybir as mybir
import concourse.tile as tile
import concourse.bass_utils as bass_utils

F32 = mybir.dt.float32
P = 128
S = 2048
D = 1024
HEADS = 4  # per core
DK = 64
E = HEADS * DK  # 256: head-group width
EO = E // P  # 2 e-subtiles
DO = D // P  # 8 d-subtiles
S_TILES = S // P  # 16
S_CHUNK = 512
S_CHUNKS = S // S_CHUNK  # 4
N_CORES = 8
SCALE = 1.0 / np.sqrt(DK)


def build_nc(debug=False):
    nc = bacc.Bacc("TRN2", target_bir_lowering=False, debug=debug,
                   num_devices=N_CORES)

    xqt = nc.dram_tensor("xqt", [D, S], F32, kind="ExternalInput")
    xkt = nc.dram_tensor("xkt", [D, S], F32, kind="ExternalInput")
    xvt = nc.dram_tensor("xvt", [D, S], F32, kind="ExternalInput")
    wqt = nc.dram_tensor("wqt", [D, E], F32, kind="ExternalInput")
    wkt = nc.dram_tensor("wkt", [D, E], F32, kind="ExternalInput")
    wvt = nc.dram_tensor("wvt", [D, E], F32, kind="ExternalInput")
    wot = nc.dram_tensor("wot", [E, D], F32, kind="ExternalInput")
    bq = nc.dram_tensor("bq", [P, EO], F32, kind="ExternalInput")
    bk = nc.dram_tensor("bk", [P, EO], F32, kind="ExternalInput")
    bv = nc.dram_tensor("bv", [P, E], F32, kind="ExternalInput")
    attn = nc.dram_tensor("attn", [HEADS, S, S], F32, kind="ExternalOutput")
    outp = nc.dram_tensor("outp", [S, D], F32, kind="ExternalOutput")

    xqt_r = xqt.ap().rearrange("(o p) s -> p o s", p=P)
    xkt_r = xkt.ap().rearrange("(o p) s -> p o s", p=P)
    xvt_r = xvt.ap().rearrange("(o p) s -> p o s", p=P)
    wqt_r = wqt.ap().rearrange("(o p) e -> p o e", p=P)
    wkt_r = wkt.ap().rearrange("(o p) e -> p o e", p=P)
    wvt_r = wvt.ap().rearrange("(o p) e -> p o e", p=P)
    wot_r = wot.ap().rearrange("(o p) f -> p o f", p=P)

    Exp = mybir.ActivationFunctionType.Exp
    Add = mybir.AluOpType.add
    Mult = mybir.AluOpType.mult

    with tile.TileContext(nc) as tc:
        with (
            tc.tile_pool(name="const", bufs=1) as const,
            tc.tile_pool(name="persist", bufs=1) as persist,
        ):
            wq_sb = const.tile([P, DO, E], F32, tag="wq")
            wk_sb = const.tile([P, DO, E], F32, tag="wk")
            wv_sb = const.tile([P, DO, E], F32, tag="wv")
            wo_sb = const.tile([P, EO, D], F32, tag="wo")
            bq_sb = const.tile([P, EO], F32, tag="bq")
            bk_sb = const.tile([P, EO], F32, tag="bk")
            bv_sb = const.tile([P, E], F32, tag="bv")
            nc.sync.dma_start(wq_sb[:], wqt_r)
            nc.sync.dma_start(wk_sb[:], wkt_r)
            nc.sync.dma_start(wv_sb[:], wvt_r)
            nc.sync.dma_start(wo_sb[:], wot_r)
            nc.sync.dma_start(bq_sb[:], bq.ap())
            nc.sync.dma_start(bk_sb[:], bk.ap())
            nc.sync.dma_start(bv_sb[:], bv.ap())

            # qht/kht: [p, eo, s] with e = eo*128 + p; head h occupies
            # e in [h*64, (h+1)*64) -> eo = h//2, partitions (h%2)*64..+64
            qht = persist.tile([P, EO, S], F32, tag="qht")
            kht = persist.tile([P, EO, S], F32, tag="kht")
            # vha: [p, s_tile, h*65 + c]; c==64 column is ones (softmax denom)
            vha = persist.tile([P, S_TILES, HEADS * (DK + 1)], F32, tag="vha")
            # ctxt: normalized context^T, same e-layout as qht
            ctxt = persist.tile([P, EO, S], F32, tag="ctxt")

            for h in range(HEADS):
                nc.vector.memset(vha[:, :, h * 65 + 64 : h * 65 + 65], 1.0)

            # ---- Phase A: projections ----
            with (
                tc.tile_pool(name="xchunk", bufs=2) as xpool,
                tc.tile_pool(name="psum_a", bufs=4, space="PSUM") as psum_a,
            ):
                for xt_r, w_sb, b_sb, out_sb in (
                    (xqt_r, wq_sb, bq_sb, qht),
                    (xkt_r, wk_sb, bk_sb, kht),
                ):
                    for sc in range(S_CHUNKS):
                        x_sb = xpool.tile([P, DO, S_CHUNK], F32, tag="xc")
                        nc.sync.dma_start(
                            x_sb[:], xt_r[:, :, sc * S_CHUNK : (sc + 1) * S_CHUNK]
                        )
                        for eo in range(EO):
                            ps = psum_a.tile([P, S_CHUNK], F32, tag="psa")
                            for d in range(DO):
                                nc.tensor.matmul(
                                    ps[:],
                                    lhsT=w_sb[:, d, eo * P : (eo + 1) * P],
                                    rhs=x_sb[:, d, :],
                                    start=(d == 0),
                                    stop=(d == DO - 1),
                                )
                            nc.vector.tensor_tensor(
                                out=out_sb[:, eo, sc * S_CHUNK : (sc + 1) * S_CHUNK],
                                in0=ps[:],
                                in1=b_sb[:, eo : eo + 1].to_broadcast((P, S_CHUNK)),
                                op=Add,
                            )
                # V: vh[s, e] layout (s on partitions)
                for sc in range(S_CHUNKS):
                    x_sb = xpool.tile([P, DO, S_CHUNK], F32, tag="xc")
                    nc.sync.dma_start(
                        x_sb[:], xvt_r[:, :, sc * S_CHUNK : (sc + 1) * S_CHUNK]
                    )
                    for st in range(S_CHUNK // P):
                        ps = psum_a.tile([P, E], F32, tag="psv")
                        for d in range(DO):
                            nc.tensor.matmul(
                                ps[:],
                                lhsT=x_sb[:, d, st * P : (st + 1) * P],
                                rhs=wv_sb[:, d, :],
                                start=(d == 0),
                                stop=(d == DO - 1),
                            )
                        t_idx = sc * (S_CHUNK // P) + st
                        nc.vector.tensor_tensor(
                            out=vha[:, t_idx].rearrange("p (h x) -> p h x", h=HEADS)[
                                :, :, 0:DK
                            ],
                            in0=ps.rearrange("p (h c) -> p h c", h=HEADS),
                            in1=bv_sb.rearrange("p (h c) -> p h c", h=HEADS),
                            op=Add,
                        )

            # ---- Phases B+C per head ----
            with (
                tc.tile_pool(name="psum_s", bufs=2, space="PSUM") as psum_s,
                tc.tile_pool(name="psum_ctx", bufs=4, space="PSUM") as psum_ctx,
                tc.tile_pool(name="attn_sb", bufs=6) as attn_pool,
                tc.tile_pool(name="expt_sb", bufs=3) as expt_pool,
                tc.tile_pool(name="small", bufs=8) as small,
            ):
                for h in range(HEADS):
                    hp = (h % 2) * DK
                    ho = h // 2
                    # B: attn output tiles [s1, s2]
                    for st in range(S_TILES):
                        at = attn_pool.tile([P, S], F32, tag="at")
                        den = small.tile([P, 2], F32, tag="den")
                        for half in range(2):
                            ps = psum_s.tile([P, S // 2], F32, tag="pss")
                            for c in range(2):
                                nc.tensor.matmul(
                                    ps[:, c * S_CHUNK : (c + 1) * S_CHUNK],
                                    lhsT=qht[hp : hp + DK, ho, st * P : (st + 1) * P],
                                    rhs=kht[
                                        hp : hp + DK,
                                        ho,
                                        (half * 2 + c) * S_CHUNK : (half * 2 + c + 1)
                                        * S_CHUNK,
                                    ],
                                    start=True,
                                    stop=True,
                                )
                            nc.scalar.activation(
                                at[:, half * (S // 2) : (half + 1) * (S // 2)],
                                ps[:],
                                Exp,
                                scale=SCALE,
                                accum_out=den[:, half : half + 1],
                            )
                        rec = small.tile([P, 1], F32, tag="rec")
                        nc.vector.reduce_sum(
                            rec[:], den[:], axis=mybir.AxisListType.X
                        )
                        nc.vector.reciprocal(rec[:], rec[:])
                        nc.vector.tensor_scalar_mul(at[:], at[:], rec[:])
                        nc.sync.dma_start(
                            attn.ap()[h, st * P : (st + 1) * P, :], at[:]
                        )
                    # C: transposed scores -> exp -> AV accumulation
                    pcs = [
                        psum_ctx.tile([DK + 1, S_CHUNK], F32, tag="pc", name=f"pc{h}_{c}")
                        for c in range(S_CHUNKS)
                    ]
                    for tt in range(S_TILES):
                        et = expt_pool.tile([P, S], F32, tag="et")
                        for half in range(2):
                            ps2 = psum_s.tile([P, S // 2], F32, tag="pss")
                            for c in range(2):
                                nc.tensor.matmul(
                                    ps2[:, c * S_CHUNK : (c + 1) * S_CHUNK],
                                    lhsT=kht[hp : hp + DK, ho, tt * P : (tt + 1) * P],
                                    rhs=qht[
                                        hp : hp + DK,
                                        ho,
                                        (half * 2 + c) * S_CHUNK : (half * 2 + c + 1)
                                        * S_CHUNK,
                                    ],
                                    start=True,
                                    stop=True,
                                )
                            nc.scalar.activation(
                                et[:, half * (S // 2) : (half + 1) * (S // 2)],
                                ps2[:],
                                Exp,
                                scale=SCALE,
                            )
                        for c in range(S_CHUNKS):
                            nc.tensor.matmul(
                                pcs[c][:],
                                lhsT=vha[:, tt, h * 65 : (h + 1) * 65],
                                rhs=et[:, c * S_CHUNK : (c + 1) * S_CHUNK],
                                start=(tt == 0),
                                stop=(tt == S_TILES - 1),
                            )
                    for c in range(S_CHUNKS):
                        rt = small.tile([1, S_CHUNK], F32, tag="rt")
                        nc.vector.reciprocal(rt[:], pcs[c][DK : DK + 1, :])
                        rbc = small.tile([DK, S_CHUNK], F32, tag="rbc")
                        nc.gpsimd.partition_broadcast(rbc[:], rt[:])
                        nc.vector.tensor_tensor(
                            out=ctxt[hp : hp + DK, ho, c * S_CHUNK : (c + 1) * S_CHUNK],
                            in0=pcs[c][0:DK, :],
                            in1=rbc[:],
                            op=Mult,
                        )

            # ---- Phase D: output projection partial ----
            with (
                tc.tile_pool(name="psum_o", bufs=4, space="PSUM") as psum_o,
                tc.tile_pool(name="out_sb", bufs=3) as out_pool,
            ):
                for st in range(S_TILES):
                    ot = out_pool.tile([P, D], F32, tag="ot")
                    for fc in range(2):
                        po = psum_o.tile([P, S_CHUNK], F32, tag="po")
                        for eo in range(EO):
                            nc.tensor.matmul(
                                po[:],
                                lhsT=ctxt[:, eo, st * P : (st + 1) * P],
                                rhs=wo_sb[:, eo, fc * S_CHUNK : (fc + 1) * S_CHUNK],
                                start=(eo == 0),
                                stop=(eo == EO - 1),
                            )
                        nc.vector.tensor_copy(
                            out=ot[:, fc * S_CHUNK : (fc + 1) * S_CHUNK], in_=po[:]
                        )
                    nc.sync.dma_start(outp.ap()[st * P : (st + 1) * P, :], ot[:])

    nc.compile()
    return nc


_NC_CACHE = {}


def _get_nc():
    if "nc" not in _NC_CACHE:
        _NC_CACHE["nc"] = build_nc(debug=False)
    return _NC_CACHE["nc"]


def make_in_maps(q, k, v, w_q, b_q, w_k, b_k, w_v, b_v, w_o, b_o):
    q = np.asarray(q, np.float32)
    k = np.asarray(k, np.float32)
    v = np.asarray(v, np.float32)
    xt = {}
    for b in range(2):
        xt[b] = (
            np.ascontiguousarray(q[b].T),
            np.ascontiguousarray(k[b].T),
            np.ascontiguousarray(v[b].T),
        )
    in_maps = []
    for core in range(N_CORES):
        b, g = core // 4, core % 4
        sl = slice(g * E, (g + 1) * E)
        in_maps.append(
            {
                "xqt": xt[b][0],
                "xkt": xt[b][1],
                "xvt": xt[b][2],
                "wqt": np.ascontiguousarray(np.asarray(w_q, np.float32)[sl, :].T),
                "wkt": np.ascontiguousarray(np.asarray(w_k, np.float32)[sl, :].T),
                "wvt": np.ascontiguousarray(np.asarray(w_v, np.float32)[sl, :].T),
                "wot": np.ascontiguousarray(np.asarray(w_o, np.float32)[:, sl].T),
                "bq": np.ascontiguousarray(
                    np.asarray(b_q, np.float32)[sl].reshape(EO, P).T
                ),
                "bk": np.ascontiguousarray(
                    np.asarray(b_k, np.float32)[sl].reshape(EO, P).T
                ),
                "bv": np.ascontiguousarray(
                    np.broadcast_to(np.asarray(b_v, np.float32)[sl], (P, E))
                ),
            }
        )
    return in_maps


def assemble_outputs(results, b_o):
    attn_full = np.empty((2, 16, S, S), np.float32)
    out_full = np.zeros((2, S, D), np.float32)
    for core in range(N_CORES):
        b, g = core // 4, core % 4
        attn_full[b, g * HEADS : (g + 1) * HEADS] = results[core]["attn"]
        out_full[b] += results[core]["outp"]
    out_full += np.asarray(b_o, np.float32)
    return out_full, attn_full


def kernel(q, k, v, w_q, b_q, w_k, b_k, w_v, b_v, w_o, b_o):
    nc = _get_nc()
    in_maps = make_in_maps(q, k, v, w_q, b_q, w_k, b_k, w_v, b_v, w_o, b_o)
    res = bass_utils.run_bass_kernel_spmd(
        nc, in_maps, core_ids=list(range(N_CORES))
    )
    return assemble_outputs(res.results, b_o)


# revision 7
# speedup vs baseline: 1.4793x; 1.4793x over previous
"""Multi-head attention (B=2, S=2048, D=1024, H=16) on 8 TRN2 NeuronCores.

Sharding: core c -> (batch b = c//4, head group g = c%4) — 4 heads/core
(tensor parallel on heads x data parallel on batch). Weight slices are
pre-transposed on the host so every device DMA is natural layout; the
w_o partial-sum reduction across each batch's 4 cores happens at gather.

Matmul operands are bf16 (PE runs fp32 as two half-rate passes — ~5x
slower); accumulation, softmax and all outputs stay fp32.
"""

import numpy as np
import ml_dtypes

import concourse.bass as bass
import concourse.bacc as bacc
import concourse.mybir as mybir
import concourse.tile as tile
import concourse.bass_utils as bass_utils

F32 = mybir.dt.float32
BF16 = mybir.dt.bfloat16
NP_BF16 = ml_dtypes.bfloat16
P = 128
S = 2048
D = 1024
HEADS = 4  # per core
DK = 64
E = HEADS * DK  # 256: head-group width
EO = E // P  # 2 e-subtiles
DO = D // P  # 8 d-subtiles
S_TILES = S // P  # 16
S_CHUNK = 512
S_CHUNKS = S // S_CHUNK  # 4
N_CORES = 8
SCALE = 1.0 / np.sqrt(DK)


def build_nc(debug=False):
    nc = bacc.Bacc("TRN2", target_bir_lowering=False, debug=debug,
                   num_devices=N_CORES)

    xqt = nc.dram_tensor("xqt", [D, S], BF16, kind="ExternalInput")
    xkt = nc.dram_tensor("xkt", [D, S], BF16, kind="ExternalInput")
    xvt = nc.dram_tensor("xvt", [D, S], BF16, kind="ExternalInput")
    wqt = nc.dram_tensor("wqt", [D, E], BF16, kind="ExternalInput")
    wkt = nc.dram_tensor("wkt", [D, E], BF16, kind="ExternalInput")
    wvt = nc.dram_tensor("wvt", [D, E], BF16, kind="ExternalInput")
    wot = nc.dram_tensor("wot", [E, D], BF16, kind="ExternalInput")
    bq = nc.dram_tensor("bq", [P, EO], F32, kind="ExternalInput")
    bk = nc.dram_tensor("bk", [P, EO], F32, kind="ExternalInput")
    bv = nc.dram_tensor("bv", [P, E], F32, kind="ExternalInput")
    attn = nc.dram_tensor("attn", [HEADS, S, S], F32, kind="ExternalOutput")
    outp = nc.dram_tensor("outp", [S, D], F32, kind="ExternalOutput")

    xqt_r = xqt.ap().rearrange("(o p) s -> p o s", p=P)
    xkt_r = xkt.ap().rearrange("(o p) s -> p o s", p=P)
    xvt_r = xvt.ap().rearrange("(o p) s -> p o s", p=P)
    wqt_r = wqt.ap().rearrange("(o p) e -> p o e", p=P)
    wkt_r = wkt.ap().rearrange("(o p) e -> p o e", p=P)
    wvt_r = wvt.ap().rearrange("(o p) e -> p o e", p=P)
    wot_r = wot.ap().rearrange("(o p) f -> p o f", p=P)

    Exp = mybir.ActivationFunctionType.Exp
    Add = mybir.AluOpType.add
    Mult = mybir.AluOpType.mult

    with tile.TileContext(nc) as tc:
        with (
            tc.tile_pool(name="const", bufs=1) as const,
            tc.tile_pool(name="persist", bufs=1) as persist,
        ):
            wq_sb = const.tile([P, DO, E], BF16, tag="wq")
            wk_sb = const.tile([P, DO, E], BF16, tag="wk")
            wv_sb = const.tile([P, DO, E], BF16, tag="wv")
            wo_sb = const.tile([P, EO, D], BF16, tag="wo")
            bq_sb = const.tile([P, EO], F32, tag="bq")
            bk_sb = const.tile([P, EO], F32, tag="bk")
            bv_sb = const.tile([P, E], F32, tag="bv")
            nc.sync.dma_start(wq_sb[:], wqt_r)
            nc.sync.dma_start(wk_sb[:], wkt_r)
            nc.sync.dma_start(wv_sb[:], wvt_r)
            nc.sync.dma_start(wo_sb[:], wot_r)
            nc.sync.dma_start(bq_sb[:], bq.ap())
            nc.sync.dma_start(bk_sb[:], bk.ap())
            nc.sync.dma_start(bv_sb[:], bv.ap())

            # qht/kht: [p, eo, s] with e = eo*128 + p; head h occupies
            # e in [h*64, (h+1)*64) -> eo = h//2, partitions (h%2)*64..+64
            qht = persist.tile([P, EO, S], BF16, tag="qht")
            kht = persist.tile([P, EO, S], BF16, tag="kht")
            # vha: [p, s_tile, h*65 + c]; c==64 column is ones (softmax denom)
            vha = persist.tile([P, S_TILES, HEADS * (DK + 1)], BF16, tag="vha")
            # ctxt: normalized context^T, same e-layout as qht
            ctxt = persist.tile([P, EO, S], BF16, tag="ctxt")

            for h in range(HEADS):
                nc.vector.memset(vha[:, :, h * 65 + 64 : h * 65 + 65], 1.0)

            # ---- Phase A: projections ----
            with (
                tc.tile_pool(name="xchunk", bufs=2) as xpool,
                tc.tile_pool(name="psum_a", bufs=4, space="PSUM") as psum_a,
            ):
                for xt_r, w_sb, b_sb, out_sb in (
                    (xqt_r, wq_sb, bq_sb, qht),
                    (xkt_r, wk_sb, bk_sb, kht),
                ):
                    for sc in range(S_CHUNKS):
                        x_sb = xpool.tile([P, DO, S_CHUNK], BF16, tag="xc")
                        nc.sync.dma_start(
                            x_sb[:], xt_r[:, :, sc * S_CHUNK : (sc + 1) * S_CHUNK]
                        )
                        for eo in range(EO):
                            ps = psum_a.tile([P, S_CHUNK], F32, tag="psa")
                            for d in range(DO):
                                nc.tensor.matmul(
                                    ps[:],
                                    lhsT=w_sb[:, d, eo * P : (eo + 1) * P],
                                    rhs=x_sb[:, d, :],
                                    start=(d == 0),
                                    stop=(d == DO - 1),
                                )
                            nc.vector.tensor_tensor(
                                out=out_sb[:, eo, sc * S_CHUNK : (sc + 1) * S_CHUNK],
                                in0=ps[:],
                                in1=b_sb[:, eo : eo + 1].to_broadcast((P, S_CHUNK)),
                                op=Add,
                            )
                # V: vh[s, e] layout (s on partitions)
                for sc in range(S_CHUNKS):
                    x_sb = xpool.tile([P, DO, S_CHUNK], BF16, tag="xc")
                    nc.sync.dma_start(
                        x_sb[:], xvt_r[:, :, sc * S_CHUNK : (sc + 1) * S_CHUNK]
                    )
                    for st in range(S_CHUNK // P):
                        ps = psum_a.tile([P, E], F32, tag="psv")
                        for d in range(DO):
                            nc.tensor.matmul(
                                ps[:],
                                lhsT=x_sb[:, d, st * P : (st + 1) * P],
                                rhs=wv_sb[:, d, :],
                                start=(d == 0),
                                stop=(d == DO - 1),
                            )
                        t_idx = sc * (S_CHUNK // P) + st
                        nc.vector.tensor_tensor(
                            out=vha[:, t_idx].rearrange("p (h x) -> p h x", h=HEADS)[
                                :, :, 0:DK
                            ],
                            in0=ps.rearrange("p (h c) -> p h c", h=HEADS),
                            in1=bv_sb.rearrange("p (h c) -> p h c", h=HEADS),
                            op=Add,
                        )

            # ---- Phases B+C per head ----
            with (
                tc.tile_pool(name="psum_s", bufs=2, space="PSUM") as psum_s,
                tc.tile_pool(name="psum_ctx", bufs=4, space="PSUM") as psum_ctx,
                tc.tile_pool(name="attn_sb", bufs=6) as attn_pool,
                tc.tile_pool(name="expt_sb", bufs=3) as expt_pool,
                tc.tile_pool(name="small", bufs=8) as small,
            ):
                for h in range(HEADS):
                    hp = (h % 2) * DK
                    ho = h // 2
                    # B: attn output tiles [s1, s2]
                    for st in range(S_TILES):
                        at = attn_pool.tile([P, S], F32, tag="at")
                        den = small.tile([P, 2], F32, tag="den")
                        for half in range(2):
                            ps = psum_s.tile([P, S // 2], F32, tag="pss")
                            for c in range(2):
                                nc.tensor.matmul(
                                    ps[:, c * S_CHUNK : (c + 1) * S_CHUNK],
                                    lhsT=qht[hp : hp + DK, ho, st * P : (st + 1) * P],
                                    rhs=kht[
                                        hp : hp + DK,
                                        ho,
                                        (half * 2 + c) * S_CHUNK : (half * 2 + c + 1)
                                        * S_CHUNK,
                                    ],
                                    start=True,
                                    stop=True,
                                )
                            nc.scalar.activation(
                                at[:, half * (S // 2) : (half + 1) * (S // 2)],
                                ps[:],
                                Exp,
                                scale=SCALE,
                                accum_out=den[:, half : half + 1],
                            )
                        rec = small.tile([P, 1], F32, tag="rec")
                        nc.vector.reduce_sum(
                            rec[:], den[:], axis=mybir.AxisListType.X
                        )
                        nc.vector.reciprocal(rec[:], rec[:])
                        nc.vector.tensor_scalar_mul(at[:], at[:], rec[:])
                        nc.sync.dma_start(
                            attn.ap()[h, st * P : (st + 1) * P, :], at[:]
                        )
                    # C: transposed scores -> exp -> AV accumulation
                    pcs = [
                        psum_ctx.tile([DK + 1, S_CHUNK], F32, tag="pc", name=f"pc{h}_{c}")
                        for c in range(S_CHUNKS)
                    ]
                    for tt in range(S_TILES):
                        et = expt_pool.tile([P, S], BF16, tag="et")
                        for half in range(2):
                            ps2 = psum_s.tile([P, S // 2], F32, tag="pss")
                            for c in range(2):
                                nc.tensor.matmul(
                                    ps2[:, c * S_CHUNK : (c + 1) * S_CHUNK],
                                    lhsT=kht[hp : hp + DK, ho, tt * P : (tt + 1) * P],
                                    rhs=qht[
                                        hp : hp + DK,
                                        ho,
                                        (half * 2 + c) * S_CHUNK : (half * 2 + c + 1)
                                        * S_CHUNK,
                                    ],
                                    start=True,
                                    stop=True,
                                )
                            nc.scalar.activation(
                                et[:, half * (S // 2) : (half + 1) * (S // 2)],
                                ps2[:],
                                Exp,
                                scale=SCALE,
                            )
                        for c in range(S_CHUNKS):
                            nc.tensor.matmul(
                                pcs[c][:],
                                lhsT=vha[:, tt, h * 65 : (h + 1) * 65],
                                rhs=et[:, c * S_CHUNK : (c + 1) * S_CHUNK],
                                start=(tt == 0),
                                stop=(tt == S_TILES - 1),
                            )
                    for c in range(S_CHUNKS):
                        rt = small.tile([1, S_CHUNK], F32, tag="rt")
                        nc.vector.reciprocal(rt[:], pcs[c][DK : DK + 1, :])
                        rbc = small.tile([DK, S_CHUNK], F32, tag="rbc")
                        nc.gpsimd.partition_broadcast(rbc[:], rt[:])
                        nc.vector.tensor_tensor(
                            out=ctxt[hp : hp + DK, ho, c * S_CHUNK : (c + 1) * S_CHUNK],
                            in0=pcs[c][0:DK, :],
                            in1=rbc[:],
                            op=Mult,
                        )

            # ---- Phase D: output projection partial ----
            with (
                tc.tile_pool(name="psum_o", bufs=4, space="PSUM") as psum_o,
                tc.tile_pool(name="out_sb", bufs=3) as out_pool,
            ):
                for st in range(S_TILES):
                    ot = out_pool.tile([P, D], F32, tag="ot")
                    for fc in range(2):
                        po = psum_o.tile([P, S_CHUNK], F32, tag="po")
                        for eo in range(EO):
                            nc.tensor.matmul(
                                po[:],
                                lhsT=ctxt[:, eo, st * P : (st + 1) * P],
                                rhs=wo_sb[:, eo, fc * S_CHUNK : (fc + 1) * S_CHUNK],
                                start=(eo == 0),
                                stop=(eo == EO - 1),
                            )
                        nc.vector.tensor_copy(
                            out=ot[:, fc * S_CHUNK : (fc + 1) * S_CHUNK], in_=po[:]
                        )
                    nc.sync.dma_start(outp.ap()[st * P : (st + 1) * P, :], ot[:])

    nc.compile()
    return nc


_NC_CACHE = {}


def _get_nc():
    if "nc" not in _NC_CACHE:
        _NC_CACHE["nc"] = build_nc(debug=False)
    return _NC_CACHE["nc"]


def make_in_maps(q, k, v, w_q, b_q, w_k, b_k, w_v, b_v, w_o, b_o):
    q = np.asarray(q, np.float32)
    k = np.asarray(k, np.float32)
    v = np.asarray(v, np.float32)
    xt = {}
    for b in range(2):
        xt[b] = (
            np.ascontiguousarray(q[b].T).astype(NP_BF16),
            np.ascontiguousarray(k[b].T).astype(NP_BF16),
            np.ascontiguousarray(v[b].T).astype(NP_BF16),
        )
    in_maps = []
    for core in range(N_CORES):
        b, g = core // 4, core % 4
        sl = slice(g * E, (g + 1) * E)
        in_maps.append(
            {
                "xqt": xt[b][0],
                "xkt": xt[b][1],
                "xvt": xt[b][2],
                "wqt": np.ascontiguousarray(
                    np.asarray(w_q, np.float32)[sl, :].T
                ).astype(NP_BF16),
                "wkt": np.ascontiguousarray(
                    np.asarray(w_k, np.float32)[sl, :].T
                ).astype(NP_BF16),
                "wvt": np.ascontiguousarray(
                    np.asarray(w_v, np.float32)[sl, :].T
                ).astype(NP_BF16),
                "wot": np.ascontiguousarray(
                    np.asarray(w_o, np.float32)[:, sl].T
                ).astype(NP_BF16),
                "bq": np.ascontiguousarray(
                    np.asarray(b_q, np.float32)[sl].reshape(EO, P).T
                ),
                "bk": np.ascontiguousarray(
                    np.asarray(b_k, np.float32)[sl].reshape(EO, P).T
                ),
                "bv": np.ascontiguousarray(
                    np.broadcast_to(np.asarray(b_v, np.float32)[sl], (P, E))
                ),
            }
        )
    return in_maps


def assemble_outputs(results, b_o):
    attn_full = np.empty((2, 16, S, S), np.float32)
    out_full = np.zeros((2, S, D), np.float32)
    for core in range(N_CORES):
        b, g = core // 4, core % 4
        attn_full[b, g * HEADS : (g + 1) * HEADS] = results[core]["attn"]
        out_full[b] += results[core]["outp"]
    out_full += np.asarray(b_o, np.float32)
    return out_full, attn_full


def kernel(q, k, v, w_q, b_q, w_k, b_k, w_v, b_v, w_o, b_o):
    nc = _get_nc()
    in_maps = make_in_maps(q, k, v, w_q, b_q, w_k, b_k, w_v, b_v, w_o, b_o)
    res = bass_utils.run_bass_kernel_spmd(
        nc, in_maps, core_ids=list(range(N_CORES))
    )
    return assemble_outputs(res.results, b_o)


# revision 14
# speedup vs baseline: 1.7715x; 1.1976x over previous
"""Multi-head attention (B=2, S=2048, D=1024, H=16) on 8 TRN2 NeuronCores.

Sharding: core c -> (batch b = c//4, head group g = c%4) — 4 heads/core
(tensor parallel on heads x data parallel on batch). Weight slices are
pre-transposed on the host so every device DMA is natural layout; the
w_o partial-sum reduction across each batch's 4 cores happens at gather.

Matmul operands are bf16 (PE runs fp32 as two half-rate passes — ~5x
slower); accumulation, softmax and all outputs stay fp32.
"""

import numpy as np
import ml_dtypes

import concourse.bass as bass
import concourse.bacc as bacc
import concourse.mybir as mybir
import concourse.tile as tile
import concourse.bass_utils as bass_utils

F32 = mybir.dt.float32
BF16 = mybir.dt.bfloat16
NP_BF16 = ml_dtypes.bfloat16
P = 128
S = 2048
D = 1024
HEADS = 4  # per core
DK = 64
E = HEADS * DK  # 256: head-group width
EO = E // P  # 2 e-subtiles
DO = D // P  # 8 d-subtiles
S_TILES = S // P  # 16
S_CHUNK = 512
S_CHUNKS = S // S_CHUNK  # 4
N_CORES = 8
SCALE = 1.0 / np.sqrt(DK)


def build_nc(debug=False):
    nc = bacc.Bacc("TRN2", target_bir_lowering=False, debug=debug,
                   num_devices=N_CORES)

    xqt = nc.dram_tensor("xqt", [D, S], BF16, kind="ExternalInput")
    xkt = nc.dram_tensor("xkt", [D, S], BF16, kind="ExternalInput")
    xvt = nc.dram_tensor("xvt", [D, S], BF16, kind="ExternalInput")
    wqt = nc.dram_tensor("wqt", [D, E], BF16, kind="ExternalInput")
    wkt = nc.dram_tensor("wkt", [D, E], BF16, kind="ExternalInput")
    wvt = nc.dram_tensor("wvt", [D, E], BF16, kind="ExternalInput")
    wot = nc.dram_tensor("wot", [E, D], BF16, kind="ExternalInput")
    bq = nc.dram_tensor("bq", [P, EO], F32, kind="ExternalInput")
    bk = nc.dram_tensor("bk", [P, EO], F32, kind="ExternalInput")
    bv = nc.dram_tensor("bv", [P, E], F32, kind="ExternalInput")
    attn = nc.dram_tensor("attn", [HEADS, S, S], F32, kind="ExternalOutput")
    outp = nc.dram_tensor("outp", [S, D], F32, kind="ExternalOutput")

    xqt_r = xqt.ap().rearrange("(o p) s -> p o s", p=P)
    xkt_r = xkt.ap().rearrange("(o p) s -> p o s", p=P)
    xvt_r = xvt.ap().rearrange("(o p) s -> p o s", p=P)
    wqt_r = wqt.ap().rearrange("(o p) e -> p o e", p=P)
    wkt_r = wkt.ap().rearrange("(o p) e -> p o e", p=P)
    wvt_r = wvt.ap().rearrange("(o p) e -> p o e", p=P)
    wot_r = wot.ap().rearrange("(o p) f -> p o f", p=P)

    Exp = mybir.ActivationFunctionType.Exp
    Add = mybir.AluOpType.add
    Mult = mybir.AluOpType.mult

    with tile.TileContext(nc) as tc:
        with (
            tc.tile_pool(name="const", bufs=1) as const,
            tc.tile_pool(name="persist", bufs=1) as persist,
        ):
            wq_sb = const.tile([P, DO, E], BF16, tag="wq")
            wk_sb = const.tile([P, DO, E], BF16, tag="wk")
            wv_sb = const.tile([P, DO, E], BF16, tag="wv")
            wo_sb = const.tile([P, EO, D], BF16, tag="wo")
            bq_sb = const.tile([P, EO], F32, tag="bq")
            bk_sb = const.tile([P, EO], F32, tag="bk")
            bv_sb = const.tile([P, E], F32, tag="bv")
            nc.sync.dma_start(wq_sb[:], wqt_r)
            nc.sync.dma_start(wk_sb[:], wkt_r)
            nc.sync.dma_start(wv_sb[:], wvt_r)
            nc.sync.dma_start(wo_sb[:], wot_r)
            nc.sync.dma_start(bq_sb[:], bq.ap())
            nc.sync.dma_start(bk_sb[:], bk.ap())
            nc.sync.dma_start(bv_sb[:], bv.ap())

            # qht/kht: [p, eo, s] with e = eo*128 + p; head h occupies
            # e in [h*64, (h+1)*64) -> eo = h//2, partitions (h%2)*64..+64
            qht = persist.tile([P, EO, S], BF16, tag="qht")
            kht = persist.tile([P, EO, S], BF16, tag="kht")
            # vha: [p, s_tile, h*64 + c]
            vha = persist.tile([P, S_TILES, E], BF16, tag="vha")
            # ctxt: normalized context^T, same e-layout as qht
            ctxt = persist.tile([P, EO, S], BF16, tag="ctxt")

            # ---- Phase A: projections ----
            with (
                tc.tile_pool(name="xchunk", bufs=2) as xpool,
                tc.tile_pool(name="psum_a", bufs=4, space="PSUM") as psum_a,
            ):
                for xt_r, w_sb, b_sb, out_sb in (
                    (xqt_r, wq_sb, bq_sb, qht),
                    (xkt_r, wk_sb, bk_sb, kht),
                ):
                    for sc in range(S_CHUNKS):
                        x_sb = xpool.tile([P, DO, S_CHUNK], BF16, tag="xc")
                        nc.sync.dma_start(
                            x_sb[:], xt_r[:, :, sc * S_CHUNK : (sc + 1) * S_CHUNK]
                        )
                        for eo in range(EO):
                            ps = psum_a.tile([P, S_CHUNK], F32, tag="psa")
                            for d in range(DO):
                                nc.tensor.matmul(
                                    ps[:],
                                    lhsT=w_sb[:, d, eo * P : (eo + 1) * P],
                                    rhs=x_sb[:, d, :],
                                    start=(d == 0),
                                    stop=(d == DO - 1),
                                )
                            nc.vector.tensor_tensor(
                                out=out_sb[:, eo, sc * S_CHUNK : (sc + 1) * S_CHUNK],
                                in0=ps[:],
                                in1=b_sb[:, eo : eo + 1].to_broadcast((P, S_CHUNK)),
                                op=Add,
                            )
                # V: vh[s, e] layout (s on partitions)
                for sc in range(S_CHUNKS):
                    x_sb = xpool.tile([P, DO, S_CHUNK], BF16, tag="xc")
                    nc.sync.dma_start(
                        x_sb[:], xvt_r[:, :, sc * S_CHUNK : (sc + 1) * S_CHUNK]
                    )
                    for st in range(S_CHUNK // P):
                        ps = psum_a.tile([P, E], F32, tag="psv")
                        for d in range(DO):
                            nc.tensor.matmul(
                                ps[:],
                                lhsT=x_sb[:, d, st * P : (st + 1) * P],
                                rhs=wv_sb[:, d, :],
                                start=(d == 0),
                                stop=(d == DO - 1),
                            )
                        t_idx = sc * (S_CHUNK // P) + st
                        nc.vector.tensor_tensor(
                            out=vha[:, t_idx, :],
                            in0=ps[:],
                            in1=bv_sb[:],
                            op=Add,
                        )

            # ---- Phases B+C: head pairs (2*ho, 2*ho+1), B and C merged ----
            with (
                tc.tile_pool(name="psum_s", bufs=2, space="PSUM") as psum_s,
                tc.tile_pool(name="psum_ctx", bufs=4, space="PSUM") as psum_ctx,
                tc.tile_pool(name="attn_sb", bufs=6) as attn_pool,
                tc.tile_pool(name="expt_sb", bufs=4) as expt_pool,
                tc.tile_pool(name="small", bufs=8) as small,
                tc.tile_pool(name="reccol", bufs=2) as reccol_pool,
                tc.tile_pool(name="dram", bufs=2, space="DRAM") as dram_pool,
            ):
                for ho in range(2):
                    # shared col-packed ctx psums: partitions hh*64..+64 = head hh
                    pcs = [
                        psum_ctx.tile([P, S_CHUNK], F32, tag="pc", name=f"pc{ho}_{c}")
                        for c in range(S_CHUNKS)
                    ]
                    rec_cols = [
                        reccol_pool.tile([P, S_TILES], F32, tag="reccol",
                                         name=f"rc{ho}_{hh}")
                        for hh in range(2)
                    ]
                    for st in range(S_TILES):
                        # B: attn output tiles [s1, s2] for both heads
                        for hh in range(2):
                            h = 2 * ho + hh
                            hp = hh * DK
                            at = attn_pool.tile([P, S], F32, tag="at")
                            den = small.tile([P, 2], F32, tag="den")
                            for half in range(2):
                                ps = psum_s.tile([P, S // 2], F32, tag="pss")
                                for c in range(2):
                                    nc.tensor.matmul(
                                        ps[:, c * S_CHUNK : (c + 1) * S_CHUNK],
                                        lhsT=qht[
                                            hp : hp + DK, ho, st * P : (st + 1) * P
                                        ],
                                        rhs=kht[
                                            hp : hp + DK,
                                            ho,
                                            (half * 2 + c)
                                            * S_CHUNK : (half * 2 + c + 1)
                                            * S_CHUNK,
                                        ],
                                        start=True,
                                        stop=True,
                                    )
                                nc.scalar.activation(
                                    at[:, half * (S // 2) : (half + 1) * (S // 2)],
                                    ps[:],
                                    Exp,
                                    scale=SCALE,
                                    accum_out=den[:, half : half + 1],
                                )
                            dsum = small.tile([P, 1], F32, tag="dsum")
                            nc.vector.reduce_sum(
                                dsum[:], den[:], axis=mybir.AxisListType.X
                            )
                            nc.vector.reciprocal(
                                rec_cols[hh][:, st : st + 1], dsum[:]
                            )
                            nc.vector.tensor_scalar_mul(
                                at[:], at[:], rec_cols[hh][:, st : st + 1]
                            )
                            nc.sync.dma_start(
                                attn.ap()[h, st * P : (st + 1) * P, :], at[:]
                            )
                        # C: transposed scores -> exp -> col-packed AV accumulation
                        ets = []
                        for hh in range(2):
                            hp = hh * DK
                            et = expt_pool.tile([P, S], BF16, tag="et",
                                                name=f"et{hh}")
                            ets.append(et)
                            for half in range(2):
                                ps2 = psum_s.tile([P, S // 2], F32, tag="pss")
                                for c in range(2):
                                    nc.tensor.matmul(
                                        ps2[:, c * S_CHUNK : (c + 1) * S_CHUNK],
                                        lhsT=kht[
                                            hp : hp + DK, ho, st * P : (st + 1) * P
                                        ],
                                        rhs=qht[
                                            hp : hp + DK,
                                            ho,
                                            (half * 2 + c)
                                            * S_CHUNK : (half * 2 + c + 1)
                                            * S_CHUNK,
                                        ],
                                        start=True,
                                        stop=True,
                                    )
                                nc.scalar.activation(
                                    et[:, half * (S // 2) : (half + 1) * (S // 2)],
                                    ps2[:],
                                    Exp,
                                    scale=SCALE,
                                )
                        for c in range(S_CHUNKS):
                            for hh in range(2):
                                h = 2 * ho + hh
                                nc.tensor.matmul(
                                    pcs[c][hh * DK : (hh + 1) * DK, :],
                                    lhsT=vha[:, st, h * DK : (h + 1) * DK],
                                    rhs=ets[hh][:, c * S_CHUNK : (c + 1) * S_CHUNK],
                                    start=(st == 0),
                                    stop=(st == S_TILES - 1),
                                    tile_position=(0, hh * DK),
                                    skip_group_check=True,
                                )
                    # normalize ctxt: recips (s1-partition layout) -> DRAM ->
                    # strided re-read as s1-major rows broadcast over partitions
                    for hh in range(2):
                        # scr[t, p] = rec[s1 = t*128+p]: s1-contiguous in DRAM
                        scr = dram_pool.tile([S_TILES, P], F32, name=f"scr{ho}_{hh}")
                        nc.sync.dma_start(
                            scr.rearrange("t p -> p t"), rec_cols[hh][:]
                        )
                        scr_flat = scr.rearrange("t p -> (t p)")
                        for c in range(S_CHUNKS):
                            rbc = small.tile([DK, S_CHUNK], F32, tag="rbc")
                            nc.sync.dma_start(
                                rbc[:],
                                scr_flat[c * S_CHUNK : (c + 1) * S_CHUNK][
                                    None
                                ].to_broadcast((DK, S_CHUNK)),
                            )
                            nc.vector.tensor_tensor(
                                out=ctxt[
                                    hh * DK : (hh + 1) * DK,
                                    ho,
                                    c * S_CHUNK : (c + 1) * S_CHUNK,
                                ],
                                in0=pcs[c][hh * DK : (hh + 1) * DK, :],
                                in1=rbc[:],
                                op=Mult,
                            )

            # ---- Phase D: output projection partial ----
            with (
                tc.tile_pool(name="psum_o", bufs=4, space="PSUM") as psum_o,
                tc.tile_pool(name="out_sb", bufs=3) as out_pool,
            ):
                for st in range(S_TILES):
                    ot = out_pool.tile([P, D], F32, tag="ot")
                    for fc in range(2):
                        po = psum_o.tile([P, S_CHUNK], F32, tag="po")
                        for eo in range(EO):
                            nc.tensor.matmul(
                                po[:],
                                lhsT=ctxt[:, eo, st * P : (st + 1) * P],
                                rhs=wo_sb[:, eo, fc * S_CHUNK : (fc + 1) * S_CHUNK],
                                start=(eo == 0),
                                stop=(eo == EO - 1),
                            )
                        nc.vector.tensor_copy(
                            out=ot[:, fc * S_CHUNK : (fc + 1) * S_CHUNK], in_=po[:]
                        )
                    nc.sync.dma_start(outp.ap()[st * P : (st + 1) * P, :], ot[:])

    nc.compile()
    return nc


_NC_CACHE = {}


def _get_nc():
    if "nc" not in _NC_CACHE:
        _NC_CACHE["nc"] = build_nc(debug=False)
    return _NC_CACHE["nc"]


def make_in_maps(q, k, v, w_q, b_q, w_k, b_k, w_v, b_v, w_o, b_o):
    q = np.asarray(q, np.float32)
    k = np.asarray(k, np.float32)
    v = np.asarray(v, np.float32)
    xt = {}
    for b in range(2):
        xt[b] = (
            np.ascontiguousarray(q[b].T).astype(NP_BF16),
            np.ascontiguousarray(k[b].T).astype(NP_BF16),
            np.ascontiguousarray(v[b].T).astype(NP_BF16),
        )
    in_maps = []
    for core in range(N_CORES):
        b, g = core // 4, core % 4
        sl = slice(g * E, (g + 1) * E)
        in_maps.append(
            {
                "xqt": xt[b][0],
                "xkt": xt[b][1],
                "xvt": xt[b][2],
                "wqt": np.ascontiguousarray(
                    np.asarray(w_q, np.float32)[sl, :].T
                ).astype(NP_BF16),
                "wkt": np.ascontiguousarray(
                    np.asarray(w_k, np.float32)[sl, :].T
                ).astype(NP_BF16),
                "wvt": np.ascontiguousarray(
                    np.asarray(w_v, np.float32)[sl, :].T
                ).astype(NP_BF16),
                "wot": np.ascontiguousarray(
                    np.asarray(w_o, np.float32)[:, sl].T
                ).astype(NP_BF16),
                "bq": np.ascontiguousarray(
                    np.asarray(b_q, np.float32)[sl].reshape(EO, P).T
                ),
                "bk": np.ascontiguousarray(
                    np.asarray(b_k, np.float32)[sl].reshape(EO, P).T
                ),
                "bv": np.ascontiguousarray(
                    np.broadcast_to(np.asarray(b_v, np.float32)[sl], (P, E))
                ),
            }
        )
    return in_maps


def assemble_outputs(results, b_o):
    attn_full = np.empty((2, 16, S, S), np.float32)
    out_full = np.zeros((2, S, D), np.float32)
    for core in range(N_CORES):
        b, g = core // 4, core % 4
        attn_full[b, g * HEADS : (g + 1) * HEADS] = results[core]["attn"]
        out_full[b] += results[core]["outp"]
    out_full += np.asarray(b_o, np.float32)
    return out_full, attn_full


def kernel(q, k, v, w_q, b_q, w_k, b_k, w_v, b_v, w_o, b_o):
    nc = _get_nc()
    in_maps = make_in_maps(q, k, v, w_q, b_q, w_k, b_k, w_v, b_v, w_o, b_o)
    res = bass_utils.run_bass_kernel_spmd(
        nc, in_maps, core_ids=list(range(N_CORES))
    )
    return assemble_outputs(res.results, b_o)


# revision 17
# speedup vs baseline: 1.8653x; 1.0529x over previous
"""Multi-head attention (B=2, S=2048, D=1024, H=16) on 8 TRN2 NeuronCores.

Sharding: core c -> (batch b = c//4, head group g = c%4) — 4 heads/core
(tensor parallel on heads x data parallel on batch). Weight slices are
pre-transposed on the host so every device DMA is natural layout; the
w_o partial-sum reduction across each batch's 4 cores happens at gather.

Matmul operands are bf16 (PE runs fp32 as two half-rate passes — ~5x
slower); accumulation, softmax and all outputs stay fp32.
"""

import numpy as np
import ml_dtypes

import concourse.bass as bass
import concourse.bacc as bacc
import concourse.mybir as mybir
import concourse.tile as tile
import concourse.bass_utils as bass_utils

F32 = mybir.dt.float32
BF16 = mybir.dt.bfloat16
NP_BF16 = ml_dtypes.bfloat16
P = 128
S = 2048
D = 1024
HEADS = 4  # per core
DK = 64
E = HEADS * DK  # 256: head-group width
EO = E // P  # 2 e-subtiles
DO = D // P  # 8 d-subtiles
S_TILES = S // P  # 16
S_CHUNK = 512
S_CHUNKS = S // S_CHUNK  # 4
N_CORES = 8
SCALE = 1.0 / np.sqrt(DK)


def build_nc(debug=False):
    nc = bacc.Bacc("TRN2", target_bir_lowering=False, debug=debug,
                   num_devices=N_CORES)

    xqt = nc.dram_tensor("xqt", [D, S], BF16, kind="ExternalInput")
    xkt = nc.dram_tensor("xkt", [D, S], BF16, kind="ExternalInput")
    xvt = nc.dram_tensor("xvt", [D, S], BF16, kind="ExternalInput")
    wqt = nc.dram_tensor("wqt", [D, E], BF16, kind="ExternalInput")
    wkt = nc.dram_tensor("wkt", [D, E], BF16, kind="ExternalInput")
    wvt = nc.dram_tensor("wvt", [D, E], BF16, kind="ExternalInput")
    wot = nc.dram_tensor("wot", [E, D], BF16, kind="ExternalInput")
    bq = nc.dram_tensor("bq", [P, EO], F32, kind="ExternalInput")
    bk = nc.dram_tensor("bk", [P, EO], F32, kind="ExternalInput")
    bv = nc.dram_tensor("bv", [P, E], F32, kind="ExternalInput")
    attn = nc.dram_tensor("attn", [HEADS, S, S], F32, kind="ExternalOutput")
    outp = nc.dram_tensor("outp", [S, D], F32, kind="ExternalOutput")

    xqt_r = xqt.ap().rearrange("(o p) s -> p o s", p=P)
    xkt_r = xkt.ap().rearrange("(o p) s -> p o s", p=P)
    xvt_r = xvt.ap().rearrange("(o p) s -> p o s", p=P)
    wqt_r = wqt.ap().rearrange("(o p) e -> p o e", p=P)
    wkt_r = wkt.ap().rearrange("(o p) e -> p o e", p=P)
    wvt_r = wvt.ap().rearrange("(o p) e -> p o e", p=P)
    wot_r = wot.ap().rearrange("(o p) f -> p o f", p=P)

    Exp = mybir.ActivationFunctionType.Exp
    Add = mybir.AluOpType.add
    Mult = mybir.AluOpType.mult

    with tile.TileContext(nc) as tc:
        with (
            tc.tile_pool(name="const", bufs=1) as const,
            tc.tile_pool(name="persist", bufs=1) as persist,
        ):
            wq_sb = const.tile([P, DO, E], BF16, tag="wq")
            wk_sb = const.tile([P, DO, E], BF16, tag="wk")
            wv_sb = const.tile([P, DO, E], BF16, tag="wv")
            wo_sb = const.tile([P, EO, D], BF16, tag="wo")
            bq_sb = const.tile([P, EO], F32, tag="bq")
            bk_sb = const.tile([P, EO], F32, tag="bk")
            bv_sb = const.tile([P, E], F32, tag="bv")
            nc.sync.dma_start(wq_sb[:], wqt_r)
            nc.sync.dma_start(wk_sb[:], wkt_r)
            nc.sync.dma_start(wv_sb[:], wvt_r)
            nc.sync.dma_start(wo_sb[:], wot_r)
            nc.sync.dma_start(bq_sb[:], bq.ap())
            nc.sync.dma_start(bk_sb[:], bk.ap())
            nc.sync.dma_start(bv_sb[:], bv.ap())

            # qht/kht: [p, eo, s] with e = eo*128 + p; head h occupies
            # e in [h*64, (h+1)*64) -> eo = h//2, partitions (h%2)*64..+64
            qht = persist.tile([P, EO, S], BF16, tag="qht")
            kht = persist.tile([P, EO, S], BF16, tag="kht")
            # vha: [p, s_tile, h*64 + c]
            vha = persist.tile([P, S_TILES, E], BF16, tag="vha")
            # ctxt: normalized context^T, same e-layout as qht
            ctxt = persist.tile([P, EO, S], BF16, tag="ctxt")

            # ---- Phase A: q/k projections (eo-outer so ho=0 unblocks first) ----
            with (
                tc.tile_pool(name="xchunk", bufs=4) as xpool,
                tc.tile_pool(name="psum_a", bufs=4, space="PSUM") as psum_a,
            ):
                xq_tiles, xk_tiles, xv_tiles = [], [], []
                for xt_r, tiles, tg in (
                    (xqt_r, xq_tiles, "xq"),
                    (xkt_r, xk_tiles, "xk"),
                    (xvt_r, xv_tiles, "xv"),
                ):
                    for sc in range(S_CHUNKS):
                        x_sb = xpool.tile([P, DO, S_CHUNK], BF16, tag=tg,
                                          name=f"{tg}{sc}")
                        nc.sync.dma_start(
                            x_sb[:], xt_r[:, :, sc * S_CHUNK : (sc + 1) * S_CHUNK]
                        )
                        tiles.append(x_sb)
                for eo in range(EO):
                    for x_tiles, w_sb, b_sb, out_sb in (
                        (xq_tiles, wq_sb, bq_sb, qht),
                        (xk_tiles, wk_sb, bk_sb, kht),
                    ):
                        for sc in range(S_CHUNKS):
                            ps = psum_a.tile([P, S_CHUNK], F32, tag="psa")
                            for d in range(DO):
                                nc.tensor.matmul(
                                    ps[:],
                                    lhsT=w_sb[:, d, eo * P : (eo + 1) * P],
                                    rhs=x_tiles[sc][:, d, :],
                                    start=(d == 0),
                                    stop=(d == DO - 1),
                                )
                            nc.vector.tensor_tensor(
                                out=out_sb[:, eo, sc * S_CHUNK : (sc + 1) * S_CHUNK],
                                in0=ps[:],
                                in1=b_sb[:, eo : eo + 1].to_broadcast((P, S_CHUNK)),
                                op=Add,
                            )
                # V: vh[s, e] layout (s on partitions)
                for sc in range(S_CHUNKS):
                    for st in range(S_CHUNK // P):
                        ps = psum_a.tile([P, E], F32, tag="psv")
                        for d in range(DO):
                            nc.tensor.matmul(
                                ps[:],
                                lhsT=xv_tiles[sc][:, d, st * P : (st + 1) * P],
                                rhs=wv_sb[:, d, :],
                                start=(d == 0),
                                stop=(d == DO - 1),
                            )
                        t_idx = sc * (S_CHUNK // P) + st
                        nc.vector.tensor_tensor(
                            out=vha[:, t_idx, :],
                            in0=ps[:],
                            in1=bv_sb[:],
                            op=Add,
                        )

            # ---- Phases B+C: head pairs (2*ho, 2*ho+1), B and C merged ----
            with (
                tc.tile_pool(name="psum_s", bufs=2, space="PSUM") as psum_s,
                tc.tile_pool(name="psum_ctx", bufs=4, space="PSUM") as psum_ctx,
                tc.tile_pool(name="attn_sb", bufs=6) as attn_pool,
                tc.tile_pool(name="expt_sb", bufs=4) as expt_pool,
                tc.tile_pool(name="small", bufs=8) as small,
                tc.tile_pool(name="reccol", bufs=2) as reccol_pool,
                tc.tile_pool(name="dram", bufs=2, space="DRAM") as dram_pool,
            ):
                for ho in range(2):
                    # shared col-packed ctx psums: partitions hh*64..+64 = head hh
                    pcs = [
                        psum_ctx.tile([P, S_CHUNK], F32, tag="pc", name=f"pc{ho}_{c}")
                        for c in range(S_CHUNKS)
                    ]
                    rec_cols = [
                        reccol_pool.tile([P, S_TILES], F32, tag="reccol",
                                         name=f"rc{ho}_{hh}")
                        for hh in range(2)
                    ]
                    for st in range(S_TILES):
                        # B: attn output tiles [s1, s2] for both heads
                        for hh in range(2):
                            h = 2 * ho + hh
                            hp = hh * DK
                            at = attn_pool.tile([P, S], F32, tag="at")
                            for half in range(2):
                                ps = psum_s.tile([P, S // 2], F32, tag="pss")
                                for c in range(2):
                                    nc.tensor.matmul(
                                        ps[:, c * S_CHUNK : (c + 1) * S_CHUNK],
                                        lhsT=qht[
                                            hp : hp + DK, ho, st * P : (st + 1) * P
                                        ],
                                        rhs=kht[
                                            hp : hp + DK,
                                            ho,
                                            (half * 2 + c)
                                            * S_CHUNK : (half * 2 + c + 1)
                                            * S_CHUNK,
                                        ],
                                        start=True,
                                        stop=True,
                                    )
                                nc.scalar.activation(
                                    at[:, half * (S // 2) : (half + 1) * (S // 2)],
                                    ps[:],
                                    Exp,
                                    scale=SCALE,
                                )
                            dsum = small.tile([P, 1], F32, tag="dsum")
                            nc.vector.reduce_sum(
                                dsum[:], at[:], axis=mybir.AxisListType.X
                            )
                            nc.vector.reciprocal(
                                rec_cols[hh][:, st : st + 1], dsum[:]
                            )
                            nc.vector.tensor_scalar_mul(
                                at[:], at[:], rec_cols[hh][:, st : st + 1]
                            )
                            nc.sync.dma_start(
                                attn.ap()[h, st * P : (st + 1) * P, :], at[:]
                            )
                        # C: transposed scores -> exp -> col-packed AV accumulation
                        ets = []
                        for hh in range(2):
                            hp = hh * DK
                            et = expt_pool.tile([P, S], BF16, tag="et",
                                                name=f"et{hh}")
                            ets.append(et)
                            for half in range(2):
                                ps2 = psum_s.tile([P, S // 2], F32, tag="pss")
                                for c in range(2):
                                    nc.tensor.matmul(
                                        ps2[:, c * S_CHUNK : (c + 1) * S_CHUNK],
                                        lhsT=kht[
                                            hp : hp + DK, ho, st * P : (st + 1) * P
                                        ],
                                        rhs=qht[
                                            hp : hp + DK,
                                            ho,
                                            (half * 2 + c)
                                            * S_CHUNK : (half * 2 + c + 1)
                                            * S_CHUNK,
                                        ],
                                        start=True,
                                        stop=True,
                                    )
                                nc.scalar.activation(
                                    et[:, half * (S // 2) : (half + 1) * (S // 2)],
                                    ps2[:],
                                    Exp,
                                    scale=SCALE,
                                )
                        for c in range(S_CHUNKS):
                            for hh in range(2):
                                h = 2 * ho + hh
                                nc.tensor.matmul(
                                    pcs[c][hh * DK : (hh + 1) * DK, :],
                                    lhsT=vha[:, st, h * DK : (h + 1) * DK],
                                    rhs=ets[hh][:, c * S_CHUNK : (c + 1) * S_CHUNK],
                                    start=(st == 0),
                                    stop=(st == S_TILES - 1),
                                    tile_position=(0, hh * DK),
                                    skip_group_check=True,
                                )
                    # normalize ctxt: recips (s1-partition layout) -> DRAM ->
                    # strided re-read as s1-major rows broadcast over partitions
                    for hh in range(2):
                        # scr[t, p] = rec[s1 = t*128+p]: s1-contiguous in DRAM
                        scr = dram_pool.tile([S_TILES, P], F32, name=f"scr{ho}_{hh}")
                        nc.sync.dma_start(
                            scr.rearrange("t p -> p t"), rec_cols[hh][:]
                        )
                        scr_flat = scr.rearrange("t p -> (t p)")
                        for c in range(S_CHUNKS):
                            rbc = small.tile([DK, S_CHUNK], F32, tag="rbc")
                            nc.sync.dma_start(
                                rbc[:],
                                scr_flat[c * S_CHUNK : (c + 1) * S_CHUNK][
                                    None
                                ].to_broadcast((DK, S_CHUNK)),
                            )
                            nc.vector.tensor_tensor(
                                out=ctxt[
                                    hh * DK : (hh + 1) * DK,
                                    ho,
                                    c * S_CHUNK : (c + 1) * S_CHUNK,
                                ],
                                in0=pcs[c][hh * DK : (hh + 1) * DK, :],
                                in1=rbc[:],
                                op=Mult,
                            )

            # ---- Phase D: output projection partial ----
            with (
                tc.tile_pool(name="psum_o", bufs=4, space="PSUM") as psum_o,
                tc.tile_pool(name="out_sb", bufs=3) as out_pool,
            ):
                for st in range(S_TILES):
                    ot = out_pool.tile([P, D], F32, tag="ot")
                    for fc in range(2):
                        po = psum_o.tile([P, S_CHUNK], F32, tag="po")
                        for eo in range(EO):
                            nc.tensor.matmul(
                                po[:],
                                lhsT=ctxt[:, eo, st * P : (st + 1) * P],
                                rhs=wo_sb[:, eo, fc * S_CHUNK : (fc + 1) * S_CHUNK],
                                start=(eo == 0),
                                stop=(eo == EO - 1),
                            )
                        nc.vector.tensor_copy(
                            out=ot[:, fc * S_CHUNK : (fc + 1) * S_CHUNK], in_=po[:]
                        )
                    nc.sync.dma_start(outp.ap()[st * P : (st + 1) * P, :], ot[:])

    nc.compile()
    return nc


_NC_CACHE = {}


def _get_nc():
    if "nc" not in _NC_CACHE:
        _NC_CACHE["nc"] = build_nc(debug=False)
    return _NC_CACHE["nc"]


def make_in_maps(q, k, v, w_q, b_q, w_k, b_k, w_v, b_v, w_o, b_o):
    q = np.asarray(q, np.float32)
    k = np.asarray(k, np.float32)
    v = np.asarray(v, np.float32)
    xt = {}
    for b in range(2):
        xt[b] = (
            np.ascontiguousarray(q[b].T).astype(NP_BF16),
            np.ascontiguousarray(k[b].T).astype(NP_BF16),
            np.ascontiguousarray(v[b].T).astype(NP_BF16),
        )
    in_maps = []
    for core in range(N_CORES):
        b, g = core // 4, core % 4
        sl = slice(g * E, (g + 1) * E)
        in_maps.append(
            {
                "xqt": xt[b][0],
                "xkt": xt[b][1],
                "xvt": xt[b][2],
                "wqt": np.ascontiguousarray(
                    np.asarray(w_q, np.float32)[sl, :].T
                ).astype(NP_BF16),
                "wkt": np.ascontiguousarray(
                    np.asarray(w_k, np.float32)[sl, :].T
                ).astype(NP_BF16),
                "wvt": np.ascontiguousarray(
                    np.asarray(w_v, np.float32)[sl, :].T
                ).astype(NP_BF16),
                "wot": np.ascontiguousarray(
                    np.asarray(w_o, np.float32)[:, sl].T
                ).astype(NP_BF16),
                "bq": np.ascontiguousarray(
                    np.asarray(b_q, np.float32)[sl].reshape(EO, P).T
                ),
                "bk": np.ascontiguousarray(
                    np.asarray(b_k, np.float32)[sl].reshape(EO, P).T
                ),
                "bv": np.ascontiguousarray(
                    np.broadcast_to(np.asarray(b_v, np.float32)[sl], (P, E))
                ),
            }
        )
    return in_maps


def assemble_outputs(results, b_o):
    attn_full = np.empty((2, 16, S, S), np.float32)
    out_full = np.zeros((2, S, D), np.float32)
    for core in range(N_CORES):
        b, g = core // 4, core % 4
        attn_full[b, g * HEADS : (g + 1) * HEADS] = results[core]["attn"]
        out_full[b] += results[core]["outp"]
    out_full += np.asarray(b_o, np.float32)
    return out_full, attn_full


def kernel(q, k, v, w_q, b_q, w_k, b_k, w_v, b_v, w_o, b_o):
    nc = _get_nc()
    in_maps = make_in_maps(q, k, v, w_q, b_q, w_k, b_k, w_v, b_v, w_o, b_o)
    res = bass_utils.run_bass_kernel_spmd(
        nc, in_maps, core_ids=list(range(N_CORES))
    )
    return assemble_outputs(res.results, b_o)
